# revision 6
# baseline (speedup 1.0000x reference)
"""Trainium2 Bass kernel for multiplicative-tril-mask attention (8 NeuronCores).

Problem: B=4, T=2048, DIN=DOUT=1024
  q = x @ Wq.T ; k = x @ Wk.T ; v = x @ Wv.T
  attn = (q @ k.T) * tril_ones        # multiplicative mask: masked logits -> 0
  attn = softmax(attn / sqrt(T))      # masked entries contribute exp(0)=1
  out = attn @ v

V3 design (one SPMD program on 8 cores, 2 cores per batch):
 - expm1 reformulation: with p~ = exp(z)-1 (masked entries -> exactly 0),
   numerator = sum_{k<win} p~ V + S0 where S0 = colsum of ALL v is a
   per-batch constant added on the HOST, and denominator = colsum(p~) + T
   (host adds the +T). No suffix matmuls; exact per-128q PV windows.
 - Even/odd query-tile assignment: parity-p core owns q-tiles
   {p, p+2, ..., p+14}; score slots pack 4 owned tiles (512 cols) with
   SPMD windows 8/16 key-tiles; PV windows are 2(s+1) for position s.
 - K/V tensor-parallel with 2-core AllGathers over DRAM bounce buffers,
   overlapped with the Q^T projection.
 - fp8e4 DoubleRow matmuls for: scores (Q^T/K^T quantized at the
   PSUM->SBUF copy), PV (p~ and V in fp8), and the V projection itself
   (x and Wv shipped as fp8 from the host). Q/K projections stay bf16.
 - Phase-B pipeline balance: gpsimd generates masks, scalar only runs
   exp, DVE does the fused (exp-1) conversion + denominator accumulate.
Measured rel err ~1.4e-2 vs the f32 reference (budget 2e-2), matching
the numpy simulation of the same quantization chain.
"""

import os
import sys

sys.path.insert(0, "/opt/trn_rl_repo")

import numpy as np
import ml_dtypes

import concourse.bass as bass
import concourse.tile as tile
from concourse import bacc, mybir
from concourse import bass_utils

bass_utils.upload_artifacts = lambda tmpdir: "local://" + tmpdir

B, T, D = 4, 2048, 1024
N_CORES = 8
NDT = D // 128
NET = D // 128
NKT_ALL = T // 128
HALF = T // 2  # 1024

NKT = [8, 16]  # score window (key tiles) per 512-query slot
SCALE = 1.0 / float(np.sqrt(np.float32(T)))

GROUPS = [[0, 1], [2, 3], [4, 5], [6, 7]]

BF = mybir.dt.bfloat16
F32 = mybir.dt.float32
FP8 = mybir.dt.float8e4
bf16 = ml_dtypes.bfloat16
f8 = ml_dtypes.float8_e4m3

_cache = {}
LAST_RESULT = None


def _dram_3d(td, dt0, ndt, c0, ncol):
    """AP over a [D, C] dram tensor viewed as [128p, ndt, ncol] starting
    at row 128*dt0, col c0 (partition = row % 128, dt = row // 128)."""
    return td.ap()[128 * dt0 : 128 * (dt0 + ndt), c0 : c0 + ncol].rearrange(
        "(dt p) c -> p dt c", dt=ndt
    )


def _build():
    nc = bacc.Bacc("TRN2", target_bir_lowering=False, debug=False, num_devices=N_CORES)

    xTh_d = nc.dram_tensor("xTh", [D, HALF], BF, kind="ExternalInput")
    xh8_d = nc.dram_tensor("xh8", [D, HALF], FP8, kind="ExternalInput")
    xTq_d = nc.dram_tensor("xTq", [D, 1024], BF, kind="ExternalInput")
    wq_d = nc.dram_tensor("wq", [D, D], BF, kind="ExternalInput")
    wk_d = nc.dram_tensor("wk", [D, D], BF, kind="ExternalInput")
    wv8_d = nc.dram_tensor("wv8", [D, D], FP8, kind="ExternalInput")
    qmi_d = nc.dram_tensor("qmi", [2, 128, 512], F32, kind="ExternalInput")
    out_d = nc.dram_tensor("out", [1024, D], BF, kind="ExternalOutput")
    den_d = nc.dram_tensor("den", [2, 512], F32, kind="ExternalOutput")

    qmi_ap = qmi_d.ap()
    out_ap = out_d.ap()

    Exp = mybir.ActivationFunctionType.Exp
    DR = mybir.MatmulPerfMode.DoubleRow

    with tile.TileContext(nc) as tc:
        with (
            tc.tile_pool(name="actpool", bufs=1) as actpool,
            tc.tile_pool(name="cpool", bufs=1) as cpool,
            tc.tile_pool(name="drpool", bufs=1, space="DRAM") as drpool,
            tc.tile_pool(name="ps_big", bufs=6, space="PSUM") as ps_big,
            tc.tile_pool(name="ps_small", bufs=2, space="PSUM") as ps_small,
        ):
            # ---- constants ----
            ones_col = cpool.tile([128, 1], BF)
            nc.vector.memset(ones_col[:], 1.0)

            qmi = cpool.tile([128, 2, 512], F32)

            # persistent activations
            QT = actpool.tile([128, NET, 1024], FP8, tag="qt")
            KT = actpool.tile([128, NET, T], FP8, tag="kt")
            V = actpool.tile([128, NKT_ALL, D], FP8, tag="v")

            # DRAM bounce buffers for collectives
            kbounce = drpool.tile([128, NET * HALF], FP8, name="kbounce")
            kg = drpool.tile([256, NET * HALF], FP8, name="kg")
            vbounce = drpool.tile([128, 8 * D], FP8, name="vbounce")
            vg = drpool.tile([256, 8 * D], FP8, name="vg")

            # ---- phase A ----
            with (
                tc.tile_pool(name="xpool", bufs=1) as xpool,
                tc.tile_pool(name="wpool", bufs=2) as wpool,
                tc.tile_pool(name="stpool", bufs=16) as stpool,
            ):
                # Coarse DMAs (one trigger per tensor-half) to cut
                # sequencer trigger serialization; wk/xh halves land first
                # since the first K group (c=0) needs only columns 0:512.
                wk_t = wpool.tile([128, NDT, D], BF, tag="w")
                xh_all = xpool.tile([128, NDT, HALF], BF, tag="xh")
                nc.sync.dma_start(wk_t[:, :, 0:512], _dram_3d(wk_d, 0, NDT, 0, 512))
                nc.scalar.dma_start(xh_all[:, :, 0:512], _dram_3d(xTh_d, 0, NDT, 0, 512))
                nc.sync.dma_start(wk_t[:, :, 512:1024], _dram_3d(wk_d, 0, NDT, 512, 512))
                nc.scalar.dma_start(
                    xh_all[:, :, 512:1024], _dram_3d(xTh_d, 0, NDT, 512, 512)
                )
                wv_t = wpool.tile([128, NDT, D], FP8, tag="w")
                xh8_all = xpool.tile([128, NDT, HALF], FP8, tag="xh8")
                nc.sync.dma_start(wv_t[:], _dram_3d(wv8_d, 0, NDT, 0, D))
                nc.scalar.dma_start(xh8_all[:], _dram_3d(xh8_d, 0, NDT, 0, HALF))
                for j in range(2):
                    nc.scalar.dma_start(qmi[:, j, :], qmi_ap[j])
                wq_t = wpool.tile([128, NDT, D], BF, tag="w")
                xq_all = xpool.tile([128, NDT, 1024], BF, tag="xq")
                nc.sync.dma_start(wq_t[:], _dram_3d(wq_d, 0, NDT, 0, D))
                nc.sync.dma_start(xq_all[:], _dram_3d(xTq_d, 0, NDT, 0, 1024))

                # K^T own half -> bounce (c outer: c=0 runs on first-half DMAs)
                for c in range(2):
                    for et in range(NET):
                        ps = ps_big.tile([128, 512], F32, tag="big", name="ps")
                        for dt in range(NDT):
                            nc.tensor.matmul(
                                ps[:],
                                wk_t[:, dt, 128 * et : 128 * (et + 1)],
                                xh_all[:, dt, 512 * c : 512 * (c + 1)],
                                start=(dt == 0),
                                stop=(dt == NDT - 1),
                            )
                        st = stpool.tile([128, 512], FP8, tag="st8", name="st8")
                        nc.vector.tensor_copy(st[:], ps[:])
                        nc.scalar.dma_start(
                            kbounce[:, HALF * et + 512 * c : HALF * et + 512 * (c + 1)],
                            st[:],
                        )
                nc.gpsimd.collective_compute(
                    "AllGather",
                    mybir.AluOpType.bypass,
                    replica_groups=GROUPS,
                    ins=[kbounce.opt()],
                    outs=[kg.opt()],
                )
                # readback gathered K^T
                for h in range(2):
                    for et in range(NET):
                        nc.sync.dma_start(
                            KT[:, et, HALF * h : HALF * (h + 1)],
                            kg[128 * h : 128 * (h + 1), HALF * et : HALF * (et + 1)],
                        )

                # V own half (8 k-tiles) -> bounce; fp8 DoubleRow over dt pairs
                for i in range(8):
                    for ec in range(2):
                        ps = ps_big.tile([128, 512], F32, tag="big", name="ps")
                        for d2 in range(NDT // 2):
                            nc.tensor.matmul(
                                ps[:],
                                xh8_all[:, 2 * d2 : 2 * d2 + 2, 128 * i : 128 * (i + 1)],
                                wv_t[:, 2 * d2 : 2 * d2 + 2, 512 * ec : 512 * (ec + 1)],
                                start=(d2 == 0),
                                stop=(d2 == NDT // 2 - 1),
                                perf_mode=DR,
                            )
                        st = stpool.tile([128, 512], FP8, tag="st8", name="st8")
                        nc.vector.tensor_copy(st[:], ps[:])
                        nc.scalar.dma_start(
                            vbounce[:, D * i + 512 * ec : D * i + 512 * (ec + 1)],
                            st[:],
                        )
                nc.gpsimd.collective_compute(
                    "AllGather",
                    mybir.AluOpType.bypass,
                    replica_groups=GROUPS,
                    ins=[vbounce.opt()],
                    outs=[vg.opt()],
                )
                for h in range(2):
                    for i in range(8):
                        nc.sync.dma_start(
                            V[:, 8 * h + i, :],
                            vg[128 * h : 128 * (h + 1), D * i : D * (i + 1)],
                        )

                # Q^T (own queries) -- fills the PE while CC(V) is in flight
                for et in range(NET):
                    for c in range(2):
                        ps = ps_big.tile([128, 512], F32, tag="big", name="ps")
                        for dt in range(NDT):
                            nc.tensor.matmul(
                                ps[:],
                                wq_t[:, dt, 128 * et : 128 * (et + 1)],
                                xq_all[:, dt, 512 * c : 512 * (c + 1)],
                                start=(dt == 0),
                                stop=(dt == NDT - 1),
                            )
                        nc.vector.tensor_copy(QT[:, et, 512 * c : 512 * (c + 1)], ps[:])

            # ---- phase B ----
            with (
                tc.tile_pool(name="ppool", bufs=2) as ppool,
                tc.tile_pool(name="epool", bufs=3) as epool,
                tc.tile_pool(name="mpool", bufs=3) as mpool,
                tc.tile_pool(name="spool", bufs=2) as spool,
                tc.tile_pool(name="opool", bufs=3) as opool,
            ):
                # bf16 accumulator for the softmax denominator
                acc = spool.tile([128, 2, 512], BF, tag="acc", name="acc", bufs=1)
                nc.vector.memset(acc[:], 0.0)

                pTs = {}
                for j in (1, 0):
                    ktj = NKT[j]
                    mask_from = 0 if j == 0 else 8

                    pT = ppool.tile([128, NKT_ALL, 512], FP8, tag="pT", name="pT")
                    pTs[j] = pT
                    for kt in range(ktj):
                        zps = ps_big.tile([128, 512], F32, tag="big", name="zps")
                        for i in range(NET // 2):
                            nc.tensor.matmul(
                                zps[:],
                                KT[:, 2 * i : 2 * i + 2, 128 * kt : 128 * (kt + 1)],
                                QT[:, 2 * i : 2 * i + 2, 512 * j : 512 * (j + 1)],
                                start=(i == 0),
                                stop=(i == NET // 2 - 1),
                                perf_mode=DR,
                            )
                        if kt >= mask_from:
                            mt = mpool.tile([128, 512], F32, tag="mask", name="mt")
                            nc.gpsimd.tensor_scalar(
                                mt[:],
                                qmi[:, j, :],
                                float(128 * kt),
                                None,
                                op0=mybir.AluOpType.is_ge,
                            )
                            nc.vector.tensor_mul(zps[:], zps[:], mt[:])
                        # e = exp(z*scale) on scalar; p~ = e - 1 lands in fp8
                        # and accumulates into den, both on DVE
                        eb = epool.tile([128, 512], BF, tag="eb", name="eb")
                        nc.scalar.activation(eb[:], zps[:], Exp, scale=SCALE)
                        nc.vector.tensor_scalar_add(pT[:, kt, :], eb[:], -1.0)
                        nc.vector.scalar_tensor_tensor(
                            acc[:, j, :],
                            eb[:],
                            -1.0,
                            acc[:, j, :],
                            op0=mybir.AluOpType.add,
                            op1=mybir.AluOpType.add,
                        )

                den_sb = spool.tile([1, 2, 512], F32, tag="den", name="den_sb", bufs=1)

                # PV with exact per-position windows (fp8 DoubleRow over kt
                # pairs), longest first so the final output block is small;
                # den matmuls slot in after the first PV group so the PE
                # doesn't stall on the DVE accumulation chain
                for s in range(7, -1, -1):
                    j, qs = s // 4, s % 4
                    win = 2 * (s + 1)
                    pT = pTs[j]
                    for ec in range(2):
                        nps = ps_big.tile([128, 512], F32, tag="big", name="nps")
                        for k2 in range(win // 2):
                            nc.tensor.matmul(
                                nps[:],
                                pT[:, 2 * k2 : 2 * k2 + 2, 128 * qs : 128 * (qs + 1)],
                                V[:, 2 * k2 : 2 * k2 + 2, 512 * ec : 512 * (ec + 1)],
                                start=(k2 == 0),
                                stop=(k2 == win // 2 - 1),
                                perf_mode=DR,
                            )
                        ot = opool.tile([128, 512], BF, tag="out", name="ot")
                        nc.vector.tensor_copy(ot[:], nps[:])
                        nc.sync.dma_start(
                            out_ap[
                                128 * s : 128 * (s + 1),
                                512 * ec : 512 * (ec + 1),
                            ],
                            ot[:],
                        )
                    if s == 7:
                        # denominator rows: den[j, q] = sum_k p~
                        for j2 in (1, 0):
                            dps = ps_small.tile(
                                [1, 512], F32, tag="small", name="dps", bufs=1
                            )
                            nc.tensor.matmul(
                                dps[:], ones_col[:], acc[:, j2, :], start=True, stop=True
                            )
                            nc.vector.tensor_copy(den_sb[:, j2, :], dps[:])
                            nc.sync.dma_start(
                                den_d.ap()[j2 : j2 + 1, :], den_sb[:, j2, :]
                            )

    nc.compile()
    return nc


def get_nc():
    if "nc" not in _cache:
        _cache["nc"] = _build()
    return _cache["nc"]


def make_in_maps(x, Wq, Wk, Wv):
    x = np.asarray(x, np.float32)
    wqT = np.ascontiguousarray(np.asarray(Wq, np.float32).T).astype(bf16)
    wkT = np.ascontiguousarray(np.asarray(Wk, np.float32).T).astype(bf16)
    wvT8 = np.ascontiguousarray(np.asarray(Wv, np.float32).T).astype(f8)

    # parity-p core owns q-tiles p, p+2, ..., p+14; slot j packs tiles
    # Tp[4j:4j+4] as 512 columns
    qmis = []
    for p in range(2):
        qmi = np.empty((2, 128, 512), np.float32)
        for j in range(2):
            gq = np.concatenate(
                [
                    128 * (p + 2 * (4 * j + c)) + np.arange(128, dtype=np.float32)
                    for c in range(4)
                ]
            )
            qmi[j] = gq[None, :] - np.arange(128, dtype=np.float32)[:, None]
        qmis.append(qmi)

    in_maps = []
    for core in range(N_CORES):
        b, p = core // 2, core % 2
        xt = np.ascontiguousarray(x[b].T)  # [D, T] f32
        xh_f = xt[:, HALF * p : HALF * (p + 1)]
        xh = np.ascontiguousarray(xh_f).astype(bf16)
        xh8 = np.ascontiguousarray(xh_f).astype(f8)
        cols = [xt[:, 128 * t : 128 * (t + 1)] for t in range(p, 16, 2)]
        xq = np.ascontiguousarray(np.concatenate(cols, axis=1)).astype(bf16)
        in_maps.append(
            {
                "xTh": xh,
                "xh8": xh8,
                "xTq": xq,
                "wq": wqT,
                "wk": wkT,
                "wv8": wvT8,
                "qmi": qmis[p],
            }
        )
    return in_maps


def assemble(x, Wv, results):
    x = np.asarray(x, np.float32)
    wv32 = np.asarray(Wv, np.float32)
    full = np.empty((B, T, D), np.float32)
    for core in range(N_CORES):
        b, p = core // 2, core % 2
        num = np.asarray(results[core]["out"], dtype=np.float32)  # [1024, D] bf16
        den = np.asarray(results[core]["den"], dtype=np.float32)  # [2, 512]
        s0 = x[b].sum(axis=0, dtype=np.float32) @ wv32.T  # [D]
        for s in range(8):
            j, qs = s // 4, s % 4
            t = p + 2 * s
            d = den[j, 128 * qs : 128 * (qs + 1)] + float(T)
            full[b, 128 * t : 128 * (t + 1), :] = (
                num[128 * s : 128 * (s + 1), :] + s0[None, :]
            ) / d[:, None]
    return full


def kernel(x, Wq, Wk, Wv):
    global LAST_RESULT
    nc = get_nc()
    in_maps = make_in_maps(x, Wq, Wk, Wv)
    res = bass_utils.run_bass_kernel_spmd(nc, in_maps, core_ids=list(range(N_CORES)))
    LAST_RESULT = res
    return assemble(x, Wv, res.results)


# revision 7
# speedup vs baseline: 2.0066x; 2.0066x over previous
"""Trainium2 Bass kernel for multiplicative-tril-mask attention (8 NeuronCores).

Problem: B=4, T=2048, DIN=DOUT=1024
  q = x @ Wq.T ; k = x @ Wk.T ; v = x @ Wv.T
  attn = (q @ k.T) * tril_ones        # multiplicative mask: masked logits -> 0
  attn = softmax(attn / sqrt(T))      # masked entries contribute exp(0)=1
  out = attn @ v

V4 design (one SPMD program on 8 cores, 2 cores per batch):
 - G-path: scores = x @ (Wq^T Wk) @ x^T. M = Wq^T Wk is host-precomputed,
   G^T = M^T x_q^T is ONE on-chip projection (replaces both Q and K
   projections), and the score lhsT is the raw fp8 x itself -- the K
   projection and its AllGather are gone entirely.
 - expm1 reformulation: p~ = exp(z)-1 (masked -> exactly 0), so
   num = sum_{k<win} p~ V + S0 with S0 = colsum(v) host-added, and
   den = colsum(p~) + T (host adds +T). Exact per-128q PV windows.
 - Even/odd query-tile assignment: parity-p core owns q-tiles
   {p, p+2, ..., p+14}; score slots pack 4 owned tiles, SPMD windows
   8/16 key-tiles; PV windows 2(s+1) for position s.
 - fp8e4 DoubleRow matmuls for the V projection (x, Wv shipped fp8),
   scores (x fp8, G quantized at the PSUM copy), and PV (p~, V fp8).
   Only the GT projection runs bf16.
 - V tensor-parallel over key halves with a single 2-core AllGather,
   launched first so it overlaps the GT projection; gpsimd runs ONLY
   the collective (masks precomputed on DVE in phase A -- gpsimd ops
   both run ~8us each and stall DVE via SBUF port contention).
 - Phase-B per key-tile: 4 DR matmuls -> exp (scalar) -> one fused DVE
   op p~ = (e-1)*mask into fp8 -> DVE den accumulate.
Measured rel err ~1.4e-2 vs the f32 reference (budget 2e-2), matching
the numpy simulation of the same quantization chain.
"""

import os
import sys

sys.path.insert(0, "/opt/trn_rl_repo")

import numpy as np
import ml_dtypes

import concourse.bass as bass
import concourse.tile as tile
from concourse import bacc, mybir
from concourse import bass_utils

bass_utils.upload_artifacts = lambda tmpdir: "local://" + tmpdir

B, T, D = 4, 2048, 1024
N_CORES = 8
NDT = D // 128
NET = D // 128
NKT_ALL = T // 128
HALF = T // 2  # 1024

NKT = [8, 16]  # score window (key tiles) per 512-query slot
SCALE = 1.0 / float(np.sqrt(np.float32(T)))

GROUPS = [[0, 1], [2, 3], [4, 5], [6, 7]]

BF = mybir.dt.bfloat16
F32 = mybir.dt.float32
FP8 = mybir.dt.float8e4
bf16 = ml_dtypes.bfloat16
f8 = ml_dtypes.float8_e4m3

_cache = {}
LAST_RESULT = None


def _dram_3d(td, dt0, ndt, c0, ncol):
    """AP over a [D, C] dram tensor viewed as [128p, ndt, ncol] starting
    at row 128*dt0, col c0 (partition = row % 128, dt = row // 128)."""
    return td.ap()[128 * dt0 : 128 * (dt0 + ndt), c0 : c0 + ncol].rearrange(
        "(dt p) c -> p dt c", dt=ndt
    )


def _build():
    nc = bacc.Bacc("TRN2", target_bir_lowering=False, debug=False, num_devices=N_CORES)

    xh8_d = nc.dram_tensor("xh8", [D, HALF], FP8, kind="ExternalInput")
    xk8_d = nc.dram_tensor("xk8", [D, T], FP8, kind="ExternalInput")
    xTq_d = nc.dram_tensor("xTq", [D, 1024], BF, kind="ExternalInput")
    m_d = nc.dram_tensor("m", [D, D], BF, kind="ExternalInput")
    wv8_d = nc.dram_tensor("wv8", [D, D], FP8, kind="ExternalInput")
    qmi_d = nc.dram_tensor("qmi", [2, 128, 512], F32, kind="ExternalInput")
    out_d = nc.dram_tensor("out", [1024, D], BF, kind="ExternalOutput")
    den_d = nc.dram_tensor("den", [2, 512], F32, kind="ExternalOutput")

    qmi_ap = qmi_d.ap()
    out_ap = out_d.ap()

    Exp = mybir.ActivationFunctionType.Exp
    DR = mybir.MatmulPerfMode.DoubleRow

    with tile.TileContext(nc) as tc:
        with (
            tc.tile_pool(name="actpool", bufs=1) as actpool,
            tc.tile_pool(name="cpool", bufs=1) as cpool,
            tc.tile_pool(name="drpool", bufs=1, space="DRAM") as drpool,
            tc.tile_pool(name="ps_big", bufs=6, space="PSUM") as ps_big,
            tc.tile_pool(name="ps_small", bufs=2, space="PSUM") as ps_small,
        ):
            # ---- constants ----
            ones_col = cpool.tile([128, 1], BF)
            nc.vector.memset(ones_col[:], 1.0)

            qmi = cpool.tile([128, 2, 512], F32)
            mk = cpool.tile([128, 16, 512], BF)  # precomputed masks per kt

            # persistent activations
            GT = actpool.tile([128, NET, 1024], FP8, tag="gt")
            XK = actpool.tile([128, NDT, T], FP8, tag="xk")
            V = actpool.tile([128, NKT_ALL, D], FP8, tag="v")

            # DRAM bounce buffers for the V collective
            vbounce = drpool.tile([128, 8 * D], FP8, name="vbounce")
            vg = drpool.tile([256, 8 * D], FP8, name="vg")

            # ---- phase A ----
            with (
                tc.tile_pool(name="xpool", bufs=1) as xpool,
                tc.tile_pool(name="wpool", bufs=1) as wpool,
                tc.tile_pool(name="stpool", bufs=16) as stpool,
            ):
                # V-projection inputs land first (it runs first so the
                # AllGather overlaps the GT projection)
                xh8_all = xpool.tile([128, NDT, HALF], FP8, tag="xh8")
                wv_t = wpool.tile([128, NDT, D], FP8, tag="wv")
                nc.scalar.dma_start(xh8_all[:], _dram_3d(xh8_d, 0, NDT, 0, HALF))
                nc.sync.dma_start(wv_t[:, :, 0:512], _dram_3d(wv8_d, 0, NDT, 0, 512))
                nc.sync.dma_start(wv_t[:, :, 512:1024], _dram_3d(wv8_d, 0, NDT, 512, 512))
                for j in range(2):
                    nc.scalar.dma_start(qmi[:, j, :], qmi_ap[j])
                m_t = wpool.tile([128, NDT, D], BF, tag="m")
                xq_all = xpool.tile([128, NDT, 1024], BF, tag="xq")
                nc.sync.dma_start(m_t[:], _dram_3d(m_d, 0, NDT, 0, D))
                nc.scalar.dma_start(xq_all[:], _dram_3d(xTq_d, 0, NDT, 0, 1024))
                nc.sync.dma_start(XK[:], _dram_3d(xk8_d, 0, NDT, 0, T))

                # V own half (8 k-tiles, fp8 DoubleRow over dt pairs) -> bounce
                for i in range(8):
                    for ec in range(2):
                        ps = ps_big.tile([128, 512], F32, tag="big", name="ps")
                        for d2 in range(NDT // 2):
                            nc.tensor.matmul(
                                ps[:],
                                xh8_all[:, 2 * d2 : 2 * d2 + 2, 128 * i : 128 * (i + 1)],
                                wv_t[:, 2 * d2 : 2 * d2 + 2, 512 * ec : 512 * (ec + 1)],
                                start=(d2 == 0),
                                stop=(d2 == NDT // 2 - 1),
                                perf_mode=DR,
                            )
                        st = stpool.tile([128, 512], FP8, tag="st8", name="st8")
                        nc.vector.tensor_copy(st[:], ps[:])
                        nc.scalar.dma_start(
                            vbounce[:, D * i + 512 * ec : D * i + 512 * (ec + 1)],
                            st[:],
                        )
                nc.gpsimd.collective_compute(
                    "AllGather",
                    mybir.AluOpType.bypass,
                    replica_groups=GROUPS,
                    ins=[vbounce.opt()],
                    outs=[vg.opt()],
                )

                # precompute masks on DVE while the PE runs projections
                for kt in range(16):
                    nc.vector.tensor_scalar(
                        mk[:, kt, :],
                        qmi[:, kt // 8, :],
                        float(128 * kt),
                        None,
                        op0=mybir.AluOpType.is_ge,
                    )

                # G^T projection (bf16): G = x @ (Wq^T Wk); lhsT = M tiles
                for et in range(NET):
                    for c in range(2):
                        ps = ps_big.tile([128, 512], F32, tag="big", name="ps")
                        for dt in range(NDT):
                            nc.tensor.matmul(
                                ps[:],
                                m_t[:, dt, 128 * et : 128 * (et + 1)],
                                xq_all[:, dt, 512 * c : 512 * (c + 1)],
                                start=(dt == 0),
                                stop=(dt == NDT - 1),
                            )
                        nc.vector.tensor_copy(GT[:, et, 512 * c : 512 * (c + 1)], ps[:])

                # V readback
                for h in range(2):
                    for i in range(8):
                        nc.sync.dma_start(
                            V[:, 8 * h + i, :],
                            vg[128 * h : 128 * (h + 1), D * i : D * (i + 1)],
                        )

            # ---- phase B ----
            with (
                tc.tile_pool(name="ppool", bufs=2) as ppool,
                tc.tile_pool(name="epool", bufs=3) as epool,
                tc.tile_pool(name="spool", bufs=2) as spool,
                tc.tile_pool(name="opool", bufs=3) as opool,
            ):
                # bf16 accumulator for the softmax denominator
                acc = spool.tile([128, 2, 512], BF, tag="acc", name="acc", bufs=1)
                nc.vector.memset(acc[:], 0.0)

                pTs = {}
                for j in (1, 0):
                    ktj = NKT[j]
                    mask_from = 0 if j == 0 else 8

                    pT = ppool.tile([128, NKT_ALL, 512], FP8, tag="pT", name="pT")
                    pTs[j] = pT
                    for kt in range(ktj):
                        zps = ps_big.tile([128, 512], F32, tag="big", name="zps")
                        for i in range(NDT // 2):
                            nc.tensor.matmul(
                                zps[:],
                                XK[:, 2 * i : 2 * i + 2, 128 * kt : 128 * (kt + 1)],
                                GT[:, 2 * i : 2 * i + 2, 512 * j : 512 * (j + 1)],
                                start=(i == 0),
                                stop=(i == NDT // 2 - 1),
                                perf_mode=DR,
                            )
                        # e = exp(z*scale) on scalar; p~ = (e-1)*mask -> fp8
                        # and den accumulate, both on DVE
                        eb = epool.tile([128, 512], BF, tag="eb", name="eb")
                        nc.scalar.activation(eb[:], zps[:], Exp, scale=SCALE)
                        if kt >= mask_from:
                            nc.vector.scalar_tensor_tensor(
                                pT[:, kt, :],
                                eb[:],
                                -1.0,
                                mk[:, kt, :],
                                op0=mybir.AluOpType.add,
                                op1=mybir.AluOpType.mult,
                            )
                        else:
                            nc.vector.tensor_scalar_add(pT[:, kt, :], eb[:], -1.0)
                        nc.vector.tensor_add(acc[:, j, :], acc[:, j, :], pT[:, kt, :])

                den_sb = spool.tile([1, 2, 512], F32, tag="den", name="den_sb", bufs=1)

                # PV with exact per-position windows (fp8 DoubleRow over kt
                # pairs), longest first so the final output block is small;
                # den matmuls slot in after the first PV group so the PE
                # doesn't stall on the DVE accumulation chain
                for s in range(7, -1, -1):
                    j, qs = s // 4, s % 4
                    win = 2 * (s + 1)
                    pT = pTs[j]
                    for ec in range(2):
                        nps = ps_big.tile([128, 512], F32, tag="big", name="nps")
                        for k2 in range(win // 2):
                            nc.tensor.matmul(
                                nps[:],
                                pT[:, 2 * k2 : 2 * k2 + 2, 128 * qs : 128 * (qs + 1)],
                                V[:, 2 * k2 : 2 * k2 + 2, 512 * ec : 512 * (ec + 1)],
                                start=(k2 == 0),
                                stop=(k2 == win // 2 - 1),
                                perf_mode=DR,
                            )
                        ot = opool.tile([128, 512], BF, tag="out", name="ot")
                        nc.vector.tensor_copy(ot[:], nps[:])
                        nc.sync.dma_start(
                            out_ap[
                                128 * s : 128 * (s + 1),
                                512 * ec : 512 * (ec + 1),
                            ],
                            ot[:],
                        )
                    if s == 7:
                        # denominator rows: den[j, q] = sum_k p~
                        for j2 in (1, 0):
                            dps = ps_small.tile(
                                [1, 512], F32, tag="small", name="dps", bufs=1
                            )
                            nc.tensor.matmul(
                                dps[:], ones_col[:], acc[:, j2, :], start=True, stop=True
                            )
                            nc.vector.tensor_copy(den_sb[:, j2, :], dps[:])
                            nc.sync.dma_start(
                                den_d.ap()[j2 : j2 + 1, :], den_sb[:, j2, :]
                            )

    nc.compile()
    return nc


def get_nc():
    if "nc" not in _cache:
        _cache["nc"] = _build()
    return _cache["nc"]


def make_in_maps(x, Wq, Wk, Wv):
    x = np.asarray(x, np.float32)
    M = (np.asarray(Wq, np.float32).T @ np.asarray(Wk, np.float32)).astype(bf16)
    wvT8 = np.ascontiguousarray(np.asarray(Wv, np.float32).T).astype(f8)

    # parity-p core owns q-tiles p, p+2, ..., p+14; slot j packs tiles
    # Tp[4j:4j+4] as 512 columns
    qmis = []
    for p in range(2):
        qmi = np.empty((2, 128, 512), np.float32)
        for j in range(2):
            gq = np.concatenate(
                [
                    128 * (p + 2 * (4 * j + c)) + np.arange(128, dtype=np.float32)
                    for c in range(4)
                ]
            )
            qmi[j] = gq[None, :] - np.arange(128, dtype=np.float32)[:, None]
        qmis.append(qmi)

    in_maps = []
    for core in range(N_CORES):
        b, p = core // 2, core % 2
        xt = np.ascontiguousarray(x[b].T)  # [D, T] f32
        xk8 = xt.astype(f8)
        xh8 = np.ascontiguousarray(xt[:, HALF * p : HALF * (p + 1)]).astype(f8)
        cols = [xt[:, 128 * t : 128 * (t + 1)] for t in range(p, 16, 2)]
        xq = np.ascontiguousarray(np.concatenate(cols, axis=1)).astype(bf16)
        in_maps.append(
            {
                "xh8": xh8,
                "xk8": xk8,
                "xTq": xq,
                "m": M,
                "wv8": wvT8,
                "qmi": qmis[p],
            }
        )
    return in_maps


def assemble(x, Wv, results):
    x = np.asarray(x, np.float32)
    wv32 = np.asarray(Wv, np.float32)
    full = np.empty((B, T, D), np.float32)
    for core in range(N_CORES):
        b, p = core // 2, core % 2
        num = np.asarray(results[core]["out"], dtype=np.float32)  # [1024, D] bf16
        den = np.asarray(results[core]["den"], dtype=np.float32)  # [2, 512]
        s0 = x[b].sum(axis=0, dtype=np.float32) @ wv32.T  # [D]
        for s in range(8):
            j, qs = s // 4, s % 4
            t = p + 2 * s
            d = den[j, 128 * qs : 128 * (qs + 1)] + float(T)
            full[b, 128 * t : 128 * (t + 1), :] = (
                num[128 * s : 128 * (s + 1), :] + s0[None, :]
            ) / d[:, None]
    return full


def kernel(x, Wq, Wk, Wv):
    global LAST_RESULT
    nc = get_nc()
    in_maps = make_in_maps(x, Wq, Wk, Wv)
    res = bass_utils.run_bass_kernel_spmd(nc, in_maps, core_ids=list(range(N_CORES)))
    LAST_RESULT = res
    return assemble(x, Wv, res.results)
